# revision 31
# baseline (speedup 1.0000x reference)
"""3-layer GAT on 8 Trainium2 NeuronCores.

Strategy (dst-sharded):
- Core k owns destination nodes [6250k, 6250(k+1)).
- Host partitions edges by dst owner, groups them into 49 blocks of 128 dst
  nodes, pads each block's edge list to whole 128-edge tiles (pad edges gather
  row 0 and carry one-hot position 255 => contribute exactly zero).
- Per layer: each core computes its slice of feat/el/er with ONE matmul using
  extended weights [W | W@al | W@ar] (el/er fold into the projection), writes
  [feat|el] rows (bf16) to a DRAM table slice, AllGathers the full table.
- Edge phase per 128-dst block: ONE batched dma_gather per (block, half)
  pulls [feat|el] rows of edge sources (int16 indices, so the 50000-row table
  is split in two halves); a one-hot matrix oh[e,d] = (dstpos[e]==d) built in
  a single compare per block both scatters (PSUM-accumulating bf16 matmul of
  [ex*feat | ex] -> [unnorm | denom]) and, transposed via TensorE, expands
  er[dst] per edge. Softmax max-subtraction is dropped (scores are O(1); the
  softmax is shift-invariant).
"""
import numpy as np

N = 50000
E = 500000
NC = 8
NLOC = N // NC          # 6250
P = 128
NBT = 49                # node tiles / blocks per core (48*128 + 106)
LAST_ROWS = NLOC - 48 * P   # 106
HALF = 32768            # int16 index split
IN = 128
D = 256
H12 = 4
F = 64
CLS = 64
TW12 = 384              # table row bf16 words (256 feat + 4 el + pad) -> 768B
TW3 = 128               # (64 feat + 1 el + pad) -> 256B
SLOPE = 0.2
NTILE0 = (N + P - 1) // P   # 391 node tiles for the local layer-0 projection
STRIP = 16


def _wrap_idx16(ix):
    """[n*128] int16 -> dma_gather wrapped layout [128, n*8]."""
    n = len(ix) // P
    return np.tile(ix.reshape(n * 8, 16).T, (8, 1)).astype(np.int16)


def make_schedule(src, dst):
    """Uniform (across cores) tile schedule + per-core index/position data."""
    src = np.asarray(src).astype(np.int64)
    dst = np.asarray(dst).astype(np.int64)
    owner = dst // NLOC
    per_core = []
    cnt = np.zeros((NC, NBT, 2), np.int64)
    for k in range(NC):
        m = owner == k
        s = src[m]
        dl = dst[m] - k * NLOC
        blk = dl // P
        pos = dl % P
        half = (s >= HALF).astype(np.int64)
        order = np.lexsort((half, blk))
        per_core.append((s[order], blk[order], pos[order], half[order]))
        for b in range(NBT):
            mb = blk[order] == b
            hb = half[order][mb]
            cnt[k, b, 0] = int((hb == 0).sum())
            cnt[k, b, 1] = int((hb == 1).sum())

    TA = np.maximum(np.ceil(cnt[:, :, 0] / P).astype(int).max(axis=0), 0)
    TB = np.maximum(np.ceil(cnt[:, :, 1] / P).astype(int).max(axis=0), 0)
    tile_block = []
    tile_half = []
    for b in range(NBT):
        tile_block += [b] * (TA[b] + TB[b])
        tile_half += [0] * TA[b] + [1] * TB[b]
    TT = len(tile_block)

    idx16 = np.zeros((NC, P, TT * 8), np.int16)
    dstpos = np.full((NC, P, TT), 255.0, np.float32)
    t0 = 0
    for b in range(NBT):
        for k in range(NC):
            s, blk, pos, half = per_core[k]
            mb = blk == b
            sb, pb, hb = s[mb], pos[mb], half[mb]
            for hv, Tn, toff in ((0, TA[b], 0), (1, TB[b], TA[b])):
                sel = hb == hv
                ss = sb[sel] - hv * HALF
                pp = pb[sel]
                nfull = len(ss)
                buf_i = np.zeros(Tn * P, np.int16)
                buf_p = np.full(Tn * P, 255.0, np.float32)
                buf_i[:nfull] = ss.astype(np.int16)
                buf_p[:nfull] = pp.astype(np.float32)
                if Tn:
                    idx16[k, :, (t0 + toff) * 8:(t0 + toff + Tn) * 8] = \
                        _wrap_idx16(buf_i)
                for j in range(Tn):
                    dstpos[k, :, t0 + toff + j] = buf_p[j * P:(j + 1) * P]
        t0 += TA[b] + TB[b]
    return tile_block, tile_half, TT, idx16, dstpos


def build_nc(tile_block, tile_half, TT, n_layers=3):
    import concourse.bacc as bacc
    import concourse.bass as bass
    import concourse.mybir as mybir
    import concourse.tile as tile
    from concourse.library_config import mlp
    dt = mybir.dt

    # per-block tile ranges
    blocks = []
    for b in range(NBT):
        blocks.append([t for t in range(len(tile_block)) if tile_block[t] == b])
    Tmax = max(len(ts) for ts in blocks)

    nc = bacc.Bacc("TRN2", target_bir_lowering=False, debug=False,
                   num_devices=NC, num_swdge_queues=4)

    xT = nc.declare_dram_parameter("xT", [IN, NBT * P], dt.bfloat16, isOutput=False)
    xTf = nc.declare_dram_parameter("xTf", [IN, NTILE0 * P], dt.bfloat16, isOutput=False)
    w1 = nc.declare_dram_parameter("w1", [IN, D + 8], dt.bfloat16, isOutput=False)
    w2 = nc.declare_dram_parameter("w2", [D, D + 8], dt.bfloat16, isOutput=False)
    w3 = nc.declare_dram_parameter("w3", [D, CLS + 2 + CLS], dt.bfloat16, isOutput=False)
    idx_in = nc.declare_dram_parameter("idx16", [P, TT * 8], dt.int16, isOutput=False)
    ohE_in = nc.declare_dram_parameter("ohE", [P, TT * P], dt.bfloat16, isOutput=False)
    ohT_in = nc.declare_dram_parameter("ohT", [P, TT * P], dt.bfloat16, isOutput=False)
    ident_in = nc.declare_dram_parameter("ident", [P, P], dt.bfloat16, isOutput=False)
    outp = nc.declare_dram_parameter("out", [NLOC, CLS], dt.float32, isOutput=True)

    slice12 = nc.dram_tensor("slice12", [NLOC, TW12], dt.bfloat16)
    table12 = nc.dram_tensor("table12", [N, TW12], dt.bfloat16, addr_space="Shared")
    slice3 = nc.dram_tensor("slice3", [NLOC, TW3], dt.bfloat16)
    table3 = nc.dram_tensor("table3", [N, TW3], dt.bfloat16, addr_space="Shared")

    groups = [list(range(NC))]

    with tile.TileContext(nc) as tc:
        with (
            tc.tile_pool(name="pers", bufs=1) as pers,
            tc.tile_pool(name="kt", bufs=3) as ktp,
            tc.tile_pool(name="stage", bufs=3) as stp,
            tc.tile_pool(name="gblk", bufs=3) as gp,
            tc.tile_pool(name="xstrip", bufs=2) as xsp,
            tc.tile_pool(name="stg", bufs=2) as stgp,
            tc.tile_pool(name="ohblk", bufs=3) as ohp,
            tc.tile_pool(name="ohT", bufs=3) as ohtp,
            tc.tile_pool(name="exR", bufs=3) as xp,
            tc.tile_pool(name="small", bufs=3) as smp,
            tc.tile_pool(name="vals", bufs=3) as vp,
            tc.tile_pool(name="otile", bufs=2) as op_,
            tc.tile_pool(name="ps_feat", bufs=2, space="PSUM") as psf,
            tc.tile_pool(name="ps_out", bufs=2, space="PSUM") as pso,
            tc.tile_pool(name="ps_tr", bufs=2, space="PSUM") as pstr,
            tc.tile_pool(name="ps_er", bufs=2, space="PSUM") as pser,
        ):
            nc.gpsimd.load_library(mlp)
            # persistent SBUF state
            xT_sb = pers.tile([P, NBT * P], dt.bfloat16)
            nc.sync.dma_start(xT_sb[:], xT[:])
            w1_sb = pers.tile([P, D + 8], dt.bfloat16)
            nc.sync.dma_start(w1_sb[:], w1[:])
            w2_sb = pers.tile([P, 2 * (D + 8)], dt.bfloat16)
            w3_sb = pers.tile([P, 2 * (CLS + 2 + CLS)], dt.bfloat16)
            for kt in range(2):
                nc.sync.dma_start(w2_sb[:, kt * (D + 8):(kt + 1) * (D + 8)],
                                  w2[kt * P:(kt + 1) * P, :])
                nc.sync.dma_start(w3_sb[:, kt * (CLS + 2 + CLS):(kt + 1) * (CLS + 2 + CLS)],
                                  w3[kt * P:(kt + 1) * P, :])
            idx_sb = pers.tile([P, TT * 8], dt.int16)
            nc.sync.dma_start(idx_sb[:], idx_in[:])
            ident_sb = pers.tile([P, P], dt.bfloat16)
            nc.sync.dma_start(ident_sb[:], ident_in[:])
            h_sb = pers.tile([P, NBT * D], dt.bfloat16)
            er_sb = pers.tile([P, NBT * H12], dt.bfloat16)
            er3_sb = pers.tile([P, NBT], dt.bfloat16)
            res_sb = pers.tile([P, NBT * CLS], dt.float32)

            tabA12 = table12[0:HALF, :]
            tabB12 = table12[HALF:N, :]
            tabA3 = table3[0:HALF, :]
            tabB3 = table3[HALF:N, :]

            qn = [0]

            def edge_phase(layer):
                if layer < 2:
                    TW, FO, NH, tabA, tabB = TW12, D, H12, tabA12, tabB12
                    er_l = er_sb
                else:
                    TW, FO, NH, tabA, tabB = TW3, CLS, 1, tabA3, tabB3
                    er_l = er3_sb
                W2c = FO + NH          # vals row width
                for b in range(NBT):
                    ts = blocks[b]
                    T = len(ts)
                    t0b = ts[0]
                    TA = sum(1 for t in ts if tile_half[t] == 0)
                    Gblk = gp.tile([P, T * TW], dt.bfloat16, tag="G")
                    if layer == 0 and b < 2:
                        nc.vector.memset(Gblk[:], 0.0)
                    ohblk = ohp.tile([P, T * P], dt.bfloat16, tag="oh")
                    ohTblk = ohtp.tile([P, T * P], dt.bfloat16, tag="ohT")
                    er_ps = pser.tile([P, T * NH], dt.float32, tag="erp")
                    for hv, toff, Tn in ((0, 0, TA), (1, TA, T - TA)):
                        if Tn == 0:
                            continue
                        nc.gpsimd.dma_gather(
                            Gblk[:, toff * TW:(toff + Tn) * TW]
                                .rearrange("p (c e) -> p c e", c=Tn),
                            tabA if hv == 0 else tabB,
                            idx_sb[:, (t0b + toff) * 8:(t0b + toff + Tn) * 8],
                            Tn * P, Tn * P, TW, queue_num=qn[0] % 4,
                        )
                        qn[0] += 1
                    nc.sync.dma_start(ohblk[:], ohE_in[:, t0b * P:(t0b + T) * P])
                    nc.sync.dma_start(ohTblk[:], ohT_in[:, t0b * P:(t0b + T) * P])
                    for j, t in enumerate(ts):
                        nc.tensor.matmul(er_ps[:, j * NH:(j + 1) * NH],
                                         ohTblk[:, j * P:(j + 1) * P],
                                         er_l[:, b * NH:(b + 1) * NH], start=True, stop=True)
                    e_sb = smp.tile([P, T * NH], dt.float32, tag="e")
                    nc.vector.tensor_tensor(
                        out=e_sb[:],
                        in0=Gblk[:].rearrange("p (t c) -> p t c", t=T)[:, :, FO:FO + NH],
                        in1=er_ps[:, :T * NH],
                        op=mybir.AluOpType.add,
                    )
                    es_sb = smp.tile([P, T * NH], dt.float32, tag="es")
                    nc.vector.tensor_scalar_mul(es_sb[:], e_sb[:], SLOPE)
                    nc.vector.tensor_tensor(out=e_sb[:], in0=e_sb[:], in1=es_sb[:],
                                            op=mybir.AluOpType.max)
                    ex_sb = smp.tile([P, T * NH], dt.float32, tag="ex")
                    nc.scalar.activation(ex_sb[:], e_sb[:], mybir.ActivationFunctionType.Exp)
                    # expand ex to the vals layout (Act engine), ones into Gblk's
                    # el columns, then one full-tile bf16 multiply (DVE 2x/4x)
                    exR = xp.tile([P, T * W2c], dt.bfloat16, tag="xr")
                    nc.scalar.activation(
                        exR[:].rearrange("p (t c) -> p t c", t=T)[:, :, 0:FO]
                            .rearrange("p t (h f) -> p t h f", h=NH),
                        ex_sb[:].rearrange("p (t h o) -> p t h o", t=T, o=1)
                            .to_broadcast([P, T, NH, F]),
                        mybir.ActivationFunctionType.Copy,
                    )
                    nc.scalar.activation(
                        exR[:].rearrange("p (t c) -> p t c", t=T)[:, :, FO:FO + NH],
                        ex_sb[:].rearrange("p (t h) -> p t h", t=T),
                        mybir.ActivationFunctionType.Copy,
                    )
                    nc.vector.memset(
                        Gblk[:].rearrange("p (t c) -> p t c", t=T)[:, :, FO:FO + NH],
                        1.0)
                    vals = vp.tile([P, T * W2c], dt.bfloat16, tag="v")
                    nc.vector.tensor_tensor(
                        out=vals[:].rearrange("p (t c) -> p t c", t=T),
                        in0=Gblk[:].rearrange("p (t c) -> p t c", t=T)[:, :, 0:W2c],
                        in1=exR[:].rearrange("p (t c) -> p t c", t=T),
                        op=mybir.AluOpType.mult,
                    )
                    out_ps = pso.tile([P, W2c], dt.float32, tag="outp")
                    for j in range(T):
                        nc.tensor.matmul(out_ps[:], ohblk[:, j * P:(j + 1) * P],
                                         vals[:, j * W2c:(j + 1) * W2c],
                                         start=(j == 0), stop=(j == T - 1))
                    den = smp.tile([P, NH], dt.float32, tag="den")
                    nc.vector.tensor_scalar_max(den[:], out_ps[:, FO:FO + NH], 1e-30)
                    rec = smp.tile([P, NH], dt.float32, tag="rec")
                    nc.vector.reciprocal(rec[:], den[:])
                    o_t = op_.tile([P, FO], dt.float32, tag="ot")
                    nc.vector.tensor_tensor(
                        out=o_t[:].rearrange("p (h f) -> p h f", h=NH),
                        in0=out_ps[:, 0:FO].rearrange("p (h f) -> p h f", h=NH),
                        in1=rec[:].to_broadcast([P, NH, F]),
                        op=mybir.AluOpType.mult,
                    )
                    # layer tails
                    if layer == 0:
                        _elu_into(o_t, h_sb, b, D)
                    elif layer == 1:
                        pre = op_.tile([P, D], dt.float32, tag="pre")
                        nc.vector.tensor_tensor(out=pre[:], in0=o_t[:],
                                                in1=h_sb[:, b * D:(b + 1) * D],
                                                op=mybir.AluOpType.add)
                        _elu_into(pre, h_sb, b, D)
                    else:
                        lg = op_.tile([P, CLS], dt.float32, tag="lg")
                        nc.vector.tensor_tensor(out=lg[:], in0=o_t[:],
                                                in1=res_sb[:, b * CLS:(b + 1) * CLS],
                                                op=mybir.AluOpType.add)
                        rows = P if b < NBT - 1 else LAST_ROWS
                        nc.sync.dma_start(outp[b * P:b * P + rows, :], lg[0:rows, :])

            def _elu_into(x_t, dst_sb, b, width):
                # elu(x) = max(x, exp(min(x,0)) - 1)
                t1 = op_.tile([P, width], dt.float32, tag="elu1")
                nc.vector.tensor_scalar_min(t1[:], x_t[:], 0.0)
                nc.scalar.activation(t1[:], t1[:], mybir.ActivationFunctionType.Exp)
                nc.vector.tensor_scalar_add(t1[:], t1[:], -1.0)
                nc.vector.tensor_tensor(out=dst_sb[:, b * width:(b + 1) * width],
                                        in0=x_t[:], in1=t1[:], op=mybir.AluOpType.max)

            def feat_phase(layer):
                if layer == 0:
                    wsb, wcols, nk = w1_sb, D + 8, 1
                elif layer == 1:
                    wsb, wcols, nk = w2_sb, D + 8, 2
                else:
                    wsb, wcols, nk = w3_sb, CLS + 2 + CLS, 2
                for nt in range(NBT):
                    f_ps = psf.tile([P, wcols], dt.float32, tag="fp")
                    for kt in range(nk):
                        if layer == 0:
                            lhsT = xT_sb[:, nt * P:(nt + 1) * P]
                        else:
                            tr_ps = pstr.tile([P, P], dt.bfloat16, tag="trp")
                            nc.tensor.transpose(
                                tr_ps[:], h_sb[:, nt * D + kt * P: nt * D + (kt + 1) * P],
                                ident_sb[:])
                            ktile = ktp.tile([P, P], dt.bfloat16, tag="kt")
                            nc.scalar.activation(ktile[:], tr_ps[:],
                                                 mybir.ActivationFunctionType.Copy)
                            lhsT = ktile[:]
                        nc.tensor.matmul(f_ps[:], lhsT, wsb[:, kt * wcols:(kt + 1) * wcols],
                                         start=(kt == 0), stop=(kt == nk - 1))
                    rows = P if nt < NBT - 1 else LAST_ROWS
                    if layer < 2:
                        st = stp.tile([P, D + H12], dt.bfloat16, tag="st")
                        nc.vector.tensor_copy(st[:], f_ps[:, 0:D + H12])
                        nc.vector.tensor_copy(er_sb[:, nt * H12:(nt + 1) * H12],
                                              f_ps[:, D + H12:D + 2 * H12])
                        nc.sync.dma_start(slice12[nt * P:nt * P + rows, 0:D + H12],
                                          st[0:rows, :])
                    else:
                        st = stp.tile([P, CLS + 1], dt.bfloat16, tag="st3")
                        nc.vector.tensor_copy(st[:], f_ps[:, 0:CLS + 1])
                        nc.vector.tensor_copy(er3_sb[:, nt:nt + 1],
                                              f_ps[:, CLS + 1:CLS + 2])
                        nc.vector.tensor_copy(res_sb[:, nt * CLS:(nt + 1) * CLS],
                                              f_ps[:, CLS + 2:CLS + 2 + CLS])
                        nc.sync.dma_start(slice3[nt * P:nt * P + rows, 0:CLS + 1],
                                          st[0:rows, :])

            def project0():
                # local er for this core's 49 blocks (wer cols of w1)
                for b in range(NBT):
                    e_ps = pser.tile([P, H12], dt.float32, tag="erp")
                    nc.tensor.matmul(e_ps[:], xT_sb[:, b * P:(b + 1) * P],
                                     w1_sb[:, D + H12:D + 2 * H12],
                                     start=True, stop=True)
                    nc.vector.tensor_copy(er_sb[:, b * H12:(b + 1) * H12], e_ps[:])
                # full-table layer-0 projection (replicated on every core)
                for st0 in range(0, NTILE0, STRIP):
                    nt_s = min(STRIP, NTILE0 - st0)
                    xs = xsp.tile([P, nt_s * P], dt.bfloat16, tag="xs")
                    nc.sync.dma_start(xs[:], xTf[:, st0 * P:(st0 + nt_s) * P])
                    stg = stgp.tile([P, nt_s * TW12], dt.bfloat16, tag="stg")
                    for a in range(nt_s):
                        f_ps = psf.tile([P, D + 8], dt.float32, tag="fp")
                        nc.tensor.matmul(f_ps[:], xs[:, a * P:(a + 1) * P], w1_sb[:],
                                         start=True, stop=True)
                        nc.vector.tensor_copy(
                            stg[:, a * TW12:a * TW12 + D + H12],
                            f_ps[:, 0:D + H12])
                    base = st0 * P
                    full = nt_s if base + nt_s * P <= N else (N - base) // P
                    if full:
                        nc.sync.dma_start(
                            table12[base:base + full * P, :]
                                .rearrange("(a p) w -> p a w", p=P),
                            stg[:, 0:full * TW12]
                                .rearrange("p (a w) -> p a w", a=full))
                    rem = (N - base) - full * P if base + nt_s * P > N else 0
                    if rem > 0:
                        nc.sync.dma_start(
                            table12[base + full * P:N, :],
                            stg[0:rem, full * TW12:(full + 1) * TW12])

            if n_layers >= 0:
                project0()
            if n_layers >= 1:
                edge_phase(0)
            for layer in range(1, n_layers):
                feat_phase(layer)
                if layer < 2:
                    nc.gpsimd.collective_compute(
                        "AllGather", mybir.AluOpType.bypass, replica_groups=groups,
                        ins=[slice12[:, :]], outs=[table12[:, :]])
                else:
                    nc.gpsimd.collective_compute(
                        "AllGather", mybir.AluOpType.bypass, replica_groups=groups,
                        ins=[slice3[:, :]], outs=[table3[:, :]])
                edge_phase(layer)

    nc.compile()
    return nc


LAST_RESULTS = None


def prepare(inputs):
    import ml_dtypes
    bf16 = ml_dtypes.bfloat16

    x = np.asarray(inputs["x"], np.float32)
    src = np.asarray(inputs["src"]).astype(np.int64)
    dst = np.asarray(inputs["dst"]).astype(np.int64)
    W1 = np.asarray(inputs["W1"], np.float32)
    W2 = np.asarray(inputs["W2"], np.float32)
    W3 = np.asarray(inputs["W3"], np.float32)
    res_W3 = np.asarray(inputs["res_W3"], np.float32)
    al1 = np.asarray(inputs["al1"], np.float32)
    ar1 = np.asarray(inputs["ar1"], np.float32)
    al2 = np.asarray(inputs["al2"], np.float32)
    ar2 = np.asarray(inputs["ar2"], np.float32)
    al3 = np.asarray(inputs["al3"], np.float32)
    ar3 = np.asarray(inputs["ar3"], np.float32)

    def ext(W, al, ar, nh, res=None):
        Wr = W.reshape(W.shape[0], nh, -1)
        wel = np.einsum("khf,hf->kh", Wr, al)
        wer = np.einsum("khf,hf->kh", Wr, ar)
        parts = [W, wel, wer] + ([res] if res is not None else [])
        return np.ascontiguousarray(np.concatenate(parts, axis=1), dtype=bf16)

    w1e = ext(W1, al1, ar1, H12)                 # [128, 264]
    w2e = ext(W2, al2, ar2, H12)                 # [256, 264]
    w3e = ext(W3, al3, ar3, 1, res_W3)           # [256, 130]

    import os
    tile_block, tile_half, TT, idx16, dstpos = make_schedule(src, dst)
    nc = build_nc(tile_block, tile_half, TT,
                  n_layers=int(os.environ.get("GAT_LAYERS", "3")))

    ident = np.eye(P, dtype=bf16)
    TT_ = len(tile_block)

    xTfull = np.pad(x.T, ((0, 0), (0, NTILE0 * P - N)))
    xTfull = np.ascontiguousarray(xTfull).astype(bf16)
    in_maps = []
    for k in range(NC):
        xk = x[k * NLOC:(k + 1) * NLOC].T                     # [128, 6250]
        xk = np.pad(xk, ((0, 0), (0, NBT * P - NLOC)))
        dp = dstpos[k]                                        # [128, TT]
        ohE = (dp[:, :, None] == np.arange(P, dtype=np.float32)[None, None, :])
        ohE = ohE.astype(bf16)                                # [e, t, d]
        ohT = np.ascontiguousarray(ohE.transpose(2, 1, 0))    # [d, t, e]
        in_maps.append({
            "xT": np.ascontiguousarray(xk).astype(bf16),
            "xTf": xTfull,
            "w1": w1e, "w2": w2e, "w3": w3e,
            "idx16": np.ascontiguousarray(idx16[k]),
            "ohE": np.ascontiguousarray(ohE.reshape(P, TT_ * P)),
            "ohT": ohT.reshape(P, TT_ * P),
            "ident": ident,
        })
    return nc, in_maps


def kernel(**inputs):
    from concourse.bass_utils import run_bass_kernel_spmd

    nc, in_maps = prepare(inputs)
    res = run_bass_kernel_spmd(nc, in_maps, core_ids=list(range(NC)))
    global LAST_RESULTS
    LAST_RESULTS = res
    out = np.concatenate([res.results[k]["out"] for k in range(NC)], axis=0)
    return out.astype(np.float32)


# revision 36
# speedup vs baseline: 1.0513x; 1.0513x over previous
"""3-layer GAT on 8 Trainium2 NeuronCores.

Strategy (dst-sharded, region-pipelined):
- Core k owns destination nodes [6250k, 6250(k+1)); edges partitioned by dst
  owner, grouped into 49 blocks of 128 dst nodes, processed in groups of 2.
- Node tables hold [feat | el | pad] rows in bf16 and are split into two
  region tensors: region 0 = local rows [0,3200) of every core, region 1 =
  [3200,6250). Region-relative gather indices stay < 25600 (int16).
- Layer 0 needs no collective: every core holds the full input (columns
  permuted to region order) and projects the whole table locally with one
  matmul per 128-node tile using extended weights [W | W@al | W@ar]; local
  er comes from per-block wer matmuls.
- Layers 1-2: feat/el/er per local node tile; each region's slice chunk is
  AllGathered separately — AG_0 is emitted mid-sweep (after block 24) and
  AG_1's transfer overlaps the next edge phase's region-0 work.
- Edge phase per 2-block group: per region ONE batched dma_gather of all
  source rows; host-precomputed one-hot tiles (edge-major ohE + transposed
  ohT, bf16) stream from DRAM; er[dst] expands per edge via ohT @ er
  matmuls; scores exponentiate on the Act engine into an expanded exR; ones
  are memset into the gathered el columns; one full-tile bf16 multiply forms
  [ex*feat | ex] which ohE^T scatter-matmuls into per-block PSUM
  accumulators ([unnorm | denom]).  Softmax max-subtraction is dropped
  (scores are O(1); softmax is shift-invariant).
"""
import numpy as np

N = 50000
E = 500000
NC = 8
NLOC = N // NC          # 6250
P = 128
NBT = 49                # dst blocks per core (48*128 + 106)
LAST_ROWS = NLOC - 48 * P   # 106
J = 2                   # table regions / sub-AllGathers per layer
CJ = [3200, 3050]       # local rows per region
OFF = [0, 3200]
RJ = [NC * CJ[0], NC * CJ[1]]   # region tensor rows: 25600, 24400
G = 1                   # edge-phase block group size
IN = 128
D = 256
H12 = 4
F = 64
CLS = 64
TW12 = 384              # table row bf16 words (256 feat + 4 el + pad) -> 768B
TW3 = 128               # (64 feat + 1 el + pad) -> 256B
SLOPE = 0.2
NTILE0 = (N + P - 1) // P   # 391 node tiles for the local layer-0 projection
STRIP = 20              # projection strip: 20 tiles = 2560 rows (region-aligned)

GROUPS = [list(range(g, min(g + G, NBT))) for g in range(0, NBT, G)]


def _wrap_idx16(ix):
    """[n*128] int16 -> dma_gather wrapped layout [128, n*8]."""
    n = len(ix) // P
    return np.tile(ix.reshape(n * 8, 16).T, (8, 1)).astype(np.int16)


def make_schedule(src, dst):
    """Uniform (across cores) tile schedule + per-core index/position data.

    Tile order: for each block-group g, for each region j, for each block b
    in g, that (b, j)'s tiles — every (g, j) is one contiguous run."""
    src = np.asarray(src).astype(np.int64)
    dst = np.asarray(dst).astype(np.int64)
    owner = dst // NLOC
    per_core = []
    cnt = np.zeros((NC, NBT, J), np.int64)
    for k in range(NC):
        m = owner == k
        s = src[m]
        dl = dst[m] - k * NLOC
        blk = dl // P
        pos = dl % P
        reg = ((s % NLOC) >= CJ[0]).astype(np.int64)
        order = np.lexsort((reg, blk))
        per_core.append((s[order], blk[order], pos[order], reg[order]))
        for b in range(NBT):
            mb = blk[order] == b
            jb = reg[order][mb]
            for j in range(J):
                cnt[k, b, j] = int((jb == j).sum())

    Tbj = np.ceil(cnt / P).astype(int).max(axis=0)       # [NBT, J]

    tile_block = []
    tile_reg = []
    for grp in GROUPS:
        for j in range(J):
            for b in grp:
                tile_block += [b] * Tbj[b, j]
                tile_reg += [j] * Tbj[b, j]
    TT = len(tile_block)

    tid = {}
    t0 = 0
    for grp in GROUPS:
        for j in range(J):
            for b in grp:
                tid[(b, j)] = (t0, Tbj[b, j])
                t0 += Tbj[b, j]

    idx16 = np.zeros((NC, P, TT * 8), np.int16)
    dstpos = np.full((NC, P, TT), 255.0, np.float32)
    for k in range(NC):
        s, blk, pos, reg = per_core[k]
        for b in range(NBT):
            for j in range(J):
                t0, Tn = tid[(b, j)]
                if Tn == 0:
                    continue
                sel = (blk == b) & (reg == j)
                ss = s[sel]
                rr = (ss // NLOC) * CJ[j] + (ss % NLOC) - OFF[j]
                pp = pos[sel]
                nfull = len(ss)
                buf_i = np.zeros(Tn * P, np.int16)
                buf_p = np.full(Tn * P, 255.0, np.float32)
                buf_i[:nfull] = rr.astype(np.int16)
                buf_p[:nfull] = pp.astype(np.float32)
                idx16[k, :, t0 * 8:(t0 + Tn) * 8] = _wrap_idx16(buf_i)
                for t in range(Tn):
                    dstpos[k, :, t0 + t] = buf_p[t * P:(t + 1) * P]
    return tile_block, tile_reg, TT, idx16, dstpos


def build_nc(tile_block, tile_reg, TT, n_layers=3):
    import concourse.bacc as bacc
    import concourse.bass as bass
    import concourse.mybir as mybir
    import concourse.tile as tile
    from concourse.library_config import mlp
    dt = mybir.dt

    # tile ranges per (b, j), per (g, j), and per g
    tid = {}
    gjr = {}
    t0 = 0
    for gi, grp in enumerate(GROUPS):
        for j in range(J):
            g0 = t0
            for b in grp:
                n = sum(1 for t in range(len(tile_block))
                        if tile_block[t] == b and tile_reg[t] == j)
                tid[(b, j)] = (t0, n)
                t0 += n
            gjr[(gi, j)] = (g0, t0 - g0)
    grange = {}
    for gi, grp in enumerate(GROUPS):
        g0 = gjr[(gi, 0)][0]
        g1 = gjr[(gi, J - 1)][0] + gjr[(gi, J - 1)][1]
        grange[gi] = (g0, g1 - g0)
    bfirst = {}
    blast = {}
    for b in range(NBT):
        ids = []
        for j in range(J):
            s0, n = tid[(b, j)]
            ids += list(range(s0, s0 + n))
        bfirst[b] = min(ids)
        blast[b] = max(ids)

    nc = bacc.Bacc("TRN2", target_bir_lowering=False, debug=False,
                   num_devices=NC, num_swdge_queues=4)

    xT = nc.declare_dram_parameter("xT", [IN, NBT * P], dt.bfloat16, isOutput=False)
    xTf = nc.declare_dram_parameter("xTf", [IN, NTILE0 * P], dt.bfloat16, isOutput=False)
    w1 = nc.declare_dram_parameter("w1", [IN, D + 8], dt.bfloat16, isOutput=False)
    w2 = nc.declare_dram_parameter("w2", [D, D + 8], dt.bfloat16, isOutput=False)
    w3 = nc.declare_dram_parameter("w3", [D, CLS + 2 + CLS], dt.bfloat16, isOutput=False)
    idx_in = nc.declare_dram_parameter("idx16", [P, TT * 8], dt.int16, isOutput=False)
    ohE_in = nc.declare_dram_parameter("ohE", [P, TT * P], dt.bfloat16, isOutput=False)
    ohT_in = nc.declare_dram_parameter("ohT", [P, TT * P], dt.bfloat16, isOutput=False)
    ident_in = nc.declare_dram_parameter("ident", [P, P], dt.bfloat16, isOutput=False)
    outp = nc.declare_dram_parameter("out", [NLOC, CLS], dt.float32, isOutput=True)

    slice12 = nc.dram_tensor("slice12", [NLOC, TW12], dt.bfloat16)
    slice3 = nc.dram_tensor("slice3", [NLOC, TW3], dt.bfloat16)
    tables = [[nc.dram_tensor(f"table_{l}_{j}", [RJ[j], TW12 if l < 2 else TW3],
                              dt.bfloat16, addr_space="Shared")
               for j in range(J)] for l in range(3)]

    groups_rg = [list(range(NC))]

    with tile.TileContext(nc) as tc:
        with (
            tc.tile_pool(name="pers", bufs=1) as pers,
            tc.tile_pool(name="kt", bufs=3) as ktp,
            tc.tile_pool(name="stage", bufs=3) as stp,
            tc.tile_pool(name="gblk", bufs=2) as gp,
            tc.tile_pool(name="xstrip", bufs=2) as xsp,
            tc.tile_pool(name="stg", bufs=2) as stgp,
            tc.tile_pool(name="ohblk", bufs=2) as ohp,
            tc.tile_pool(name="exR", bufs=2) as xp,
            tc.tile_pool(name="small", bufs=3) as smp,
            tc.tile_pool(name="vals", bufs=2) as vp,
            tc.tile_pool(name="otile", bufs=2) as op_,
            tc.tile_pool(name="ps_feat", bufs=2, space="PSUM") as psf,
            tc.tile_pool(name="ps_out", bufs=2, space="PSUM") as pso,
            tc.tile_pool(name="ps_tr", bufs=2, space="PSUM") as pstr,
            tc.tile_pool(name="ps_er", bufs=2, space="PSUM") as pser,
        ):
            nc.gpsimd.load_library(mlp)
            # persistent SBUF state
            xT_sb = pers.tile([P, NBT * P], dt.bfloat16)
            nc.sync.dma_start(xT_sb[:], xT[:])
            w1_sb = pers.tile([P, D + 8], dt.bfloat16)
            nc.sync.dma_start(w1_sb[:], w1[:])
            w2_sb = pers.tile([P, 2 * (D + 8)], dt.bfloat16)
            w3_sb = pers.tile([P, 2 * (CLS + 2 + CLS)], dt.bfloat16)
            for kt in range(2):
                nc.sync.dma_start(w2_sb[:, kt * (D + 8):(kt + 1) * (D + 8)],
                                  w2[kt * P:(kt + 1) * P, :])
                nc.sync.dma_start(w3_sb[:, kt * (CLS + 2 + CLS):(kt + 1) * (CLS + 2 + CLS)],
                                  w3[kt * P:(kt + 1) * P, :])
            idx_sb = pers.tile([P, TT * 8], dt.int16)
            nc.sync.dma_start(idx_sb[:], idx_in[:])
            ident_sb = pers.tile([P, P], dt.bfloat16)
            nc.sync.dma_start(ident_sb[:], ident_in[:])
            h_sb = pers.tile([P, NBT * D], dt.bfloat16)
            er_sb = pers.tile([P, NBT * H12], dt.bfloat16)
            er3_sb = pers.tile([P, NBT], dt.bfloat16)
            res_sb = pers.tile([P, NBT * CLS], dt.float32)

            def _elu_into(x_t, dst_sb, b, width):
                # elu(x) = max(x, exp(min(x,0)) - 1)
                t1 = op_.tile([P, width], dt.float32, tag="elu1")
                nc.vector.tensor_scalar_min(t1[:], x_t[:], 0.0)
                nc.scalar.activation(t1[:], t1[:], mybir.ActivationFunctionType.Exp)
                nc.vector.tensor_scalar_add(t1[:], t1[:], -1.0)
                nc.vector.tensor_tensor(out=dst_sb[:, b * width:(b + 1) * width],
                                        in0=x_t[:], in1=t1[:], op=mybir.AluOpType.max)

            def project0():
                # local er for this core's 49 blocks (wer cols of w1)
                for b in range(NBT):
                    e_ps = pser.tile([P, H12], dt.float32, tag="erp")
                    nc.tensor.matmul(e_ps[:], xT_sb[:, b * P:(b + 1) * P],
                                     w1_sb[:, D + H12:D + 2 * H12],
                                     start=True, stop=True)
                    nc.vector.tensor_copy(er_sb[:, b * H12:(b + 1) * H12], e_ps[:])
                # full-table layer-0 projection (replicated on every core);
                # xTf columns are pre-permuted to region-major table order
                for st0 in range(0, NTILE0, STRIP):
                    nt_s = min(STRIP, NTILE0 - st0)
                    xs = xsp.tile([P, nt_s * P], dt.bfloat16, tag="xs")
                    nc.sync.dma_start(xs[:], xTf[:, st0 * P:(st0 + nt_s) * P])
                    stg = stgp.tile([P, nt_s * TW12], dt.bfloat16, tag="stg")
                    for a in range(nt_s):
                        f_ps = psf.tile([P, D + 8], dt.float32, tag="fp")
                        nc.tensor.matmul(f_ps[:], xs[:, a * P:(a + 1) * P], w1_sb[:],
                                         start=True, stop=True)
                        nc.vector.tensor_copy(
                            stg[:, a * TW12:a * TW12 + D + H12],
                            f_ps[:, 0:D + H12])
                    base = st0 * P
                    if base < RJ[0]:
                        tab, lbase, lim = tables[0][0], base, RJ[0]
                    else:
                        tab, lbase, lim = tables[0][1], base - RJ[0], N - RJ[0]
                    full = nt_s if lbase + nt_s * P <= lim else (lim - lbase) // P
                    if full:
                        nc.sync.dma_start(
                            tab[lbase:lbase + full * P, :]
                                .rearrange("(a p) w -> p a w", p=P),
                            stg[:, 0:full * TW12]
                                .rearrange("p (a w) -> p a w", a=full))
                    rem = (lim - lbase) - full * P if lbase + nt_s * P > lim else 0
                    if rem > 0:
                        nc.sync.dma_start(
                            tab[lbase + full * P:lim, :],
                            stg[0:rem, full * TW12:(full + 1) * TW12])

            def feat_blocks(layer, b0, b1):
                if layer == 1:
                    wsb, wcols = w2_sb, D + 8
                else:
                    wsb, wcols = w3_sb, CLS + 2 + CLS
                for nt in range(b0, b1):
                    f_ps = psf.tile([P, wcols], dt.float32, tag="fp")
                    for kt in range(2):
                        tr_ps = pstr.tile([P, P], dt.bfloat16, tag="trp")
                        nc.tensor.transpose(
                            tr_ps[:], h_sb[:, nt * D + kt * P: nt * D + (kt + 1) * P],
                            ident_sb[:])
                        ktile = ktp.tile([P, P], dt.bfloat16, tag="kt")
                        nc.scalar.activation(ktile[:], tr_ps[:],
                                             mybir.ActivationFunctionType.Copy)
                        nc.tensor.matmul(f_ps[:], ktile[:],
                                         wsb[:, kt * wcols:(kt + 1) * wcols],
                                         start=(kt == 0), stop=(kt == 1))
                    rows = P if nt < NBT - 1 else LAST_ROWS
                    if layer < 2:
                        st = stp.tile([P, D + H12], dt.bfloat16, tag="st")
                        nc.vector.tensor_copy(st[:], f_ps[:, 0:D + H12])
                        nc.vector.tensor_copy(er_sb[:, nt * H12:(nt + 1) * H12],
                                              f_ps[:, D + H12:D + 2 * H12])
                        nc.sync.dma_start(slice12[nt * P:nt * P + rows, 0:D + H12],
                                          st[0:rows, :])
                    else:
                        st = stp.tile([P, CLS + 1], dt.bfloat16, tag="st3")
                        nc.vector.tensor_copy(st[:], f_ps[:, 0:CLS + 1])
                        nc.vector.tensor_copy(er3_sb[:, nt:nt + 1],
                                              f_ps[:, CLS + 1:CLS + 2])
                        nc.vector.tensor_copy(res_sb[:, nt * CLS:(nt + 1) * CLS],
                                              f_ps[:, CLS + 2:CLS + 2 + CLS])
                        nc.sync.dma_start(slice3[nt * P:nt * P + rows, 0:CLS + 1],
                                          st[0:rows, :])

            def emit_ag(layer, j):
                sl = slice12 if layer < 2 else slice3
                nc.gpsimd.collective_compute(
                    "AllGather", mybir.AluOpType.bypass, replica_groups=groups_rg,
                    ins=[sl[OFF[j]:OFF[j] + CJ[j], :]],
                    outs=[tables[layer][j][:, :]])

            qn = [0]

            def edge_phase(layer):
                if layer < 2:
                    TW, FO, NH = TW12, D, H12
                    er_l = er_sb
                else:
                    TW, FO, NH = TW3, CLS, 1
                    er_l = er3_sb
                W2c = FO + NH          # vals row width
                for gi, grp in enumerate(GROUPS):
                    gt0, gT = grange[gi]
                    ohE_g = ohp.tile([P, gT * P], dt.bfloat16, tag="ohE")
                    ohT_g = ohp.tile([P, gT * P], dt.bfloat16, tag="ohT")
                    nc.sync.dma_start(ohE_g[:], ohE_in[:, gt0 * P:(gt0 + gT) * P])
                    nc.sync.dma_start(ohT_g[:], ohT_in[:, gt0 * P:(gt0 + gT) * P])
                    outps = {}
                    for b in grp:
                        out_acc = pso.tile([P, W2c], dt.float32, tag="outp",
                                           name=f"outacc_{layer}_{gi}_{b}")
                        outps[b] = out_acc
                    vals_j = {}
                    for j in range(J):
                        j0, jT = gjr[(gi, j)]
                        if jT == 0:
                            continue
                        Gblk = gp.tile([P, jT * TW], dt.bfloat16, tag="G")
                        nc.gpsimd.dma_gather(
                            Gblk[:].rearrange("p (c e) -> p c e", c=jT),
                            tables[layer][j][:, :],
                            idx_sb[:, j0 * 8:(j0 + jT) * 8],
                            jT * P, jT * P, TW, queue_num=qn[0] % 4,
                        )
                        qn[0] += 1
                        er_ps = pser.tile([P, jT * NH], dt.float32, tag="erp")
                        for b in grp:
                            s0, n = tid[(b, j)]
                            for t in range(s0, s0 + n):
                                lo = t - gt0
                                nc.tensor.matmul(
                                    er_ps[:, (t - j0) * NH:(t - j0 + 1) * NH],
                                    ohT_g[:, lo * P:(lo + 1) * P],
                                    er_l[:, b * NH:(b + 1) * NH],
                                    start=True, stop=True)
                        e_sb = smp.tile([P, jT * NH], dt.float32, tag="e")
                        nc.vector.tensor_tensor(
                            out=e_sb[:],
                            in0=Gblk[:].rearrange("p (t c) -> p t c", t=jT)[:, :, FO:FO + NH],
                            in1=er_ps[:, :jT * NH],
                            op=mybir.AluOpType.add,
                        )
                        es_sb = smp.tile([P, jT * NH], dt.float32, tag="es")
                        nc.vector.tensor_scalar_mul(es_sb[:], e_sb[:], SLOPE)
                        nc.vector.tensor_tensor(out=e_sb[:], in0=e_sb[:], in1=es_sb[:],
                                                op=mybir.AluOpType.max)
                        ex_sb = smp.tile([P, jT * NH], dt.float32, tag="ex")
                        nc.scalar.activation(ex_sb[:], e_sb[:],
                                             mybir.ActivationFunctionType.Exp)
                        exR = xp.tile([P, jT * W2c], dt.bfloat16, tag="xr")
                        nc.scalar.activation(
                            exR[:].rearrange("p (t c) -> p t c", t=jT)[:, :, 0:FO]
                                .rearrange("p t (h f) -> p t h f", h=NH),
                            ex_sb[:].rearrange("p (t h o) -> p t h o", t=jT, o=1)
                                .to_broadcast([P, jT, NH, F]),
                            mybir.ActivationFunctionType.Copy,
                        )
                        nc.scalar.activation(
                            exR[:].rearrange("p (t c) -> p t c", t=jT)[:, :, FO:FO + NH],
                            ex_sb[:].rearrange("p (t h) -> p t h", t=jT),
                            mybir.ActivationFunctionType.Copy,
                        )
                        nc.vector.memset(
                            Gblk[:].rearrange("p (t c) -> p t c", t=jT)[:, :, FO:FO + NH],
                            1.0)
                        vals = vp.tile([P, jT * W2c], dt.bfloat16, tag="v")
                        nc.vector.tensor_tensor(
                            out=vals[:].rearrange("p (t c) -> p t c", t=jT),
                            in0=Gblk[:].rearrange("p (t c) -> p t c", t=jT)[:, :, 0:W2c],
                            in1=exR[:].rearrange("p (t c) -> p t c", t=jT),
                            op=mybir.AluOpType.mult,
                        )
                        vals_j[j] = (vals, j0)
                    # contiguous PSUM accumulation group per block
                    for b in grp:
                        for j in range(J):
                            if j not in vals_j:
                                continue
                            vals, j0 = vals_j[j]
                            s0, n = tid[(b, j)]
                            for t in range(s0, s0 + n):
                                lo = t - gt0
                                nc.tensor.matmul(
                                    outps[b][:],
                                    ohE_g[:, lo * P:(lo + 1) * P],
                                    vals[:, (t - j0) * W2c:(t - j0 + 1) * W2c],
                                    start=(t == bfirst[b]), stop=(t == blast[b]))
                    for b in grp:
                        out_ps = outps[b]
                        den = smp.tile([P, NH], dt.float32, tag="den")
                        nc.vector.tensor_scalar_max(den[:], out_ps[:, FO:FO + NH], 1e-30)
                        rec = smp.tile([P, NH], dt.float32, tag="rec")
                        nc.vector.reciprocal(rec[:], den[:])
                        o_t = op_.tile([P, FO], dt.float32, tag="ot")
                        nc.vector.tensor_tensor(
                            out=o_t[:].rearrange("p (h f) -> p h f", h=NH),
                            in0=out_ps[:, 0:FO].rearrange("p (h f) -> p h f", h=NH),
                            in1=rec[:].to_broadcast([P, NH, F]),
                            op=mybir.AluOpType.mult,
                        )
                        # layer tails
                        if layer == 0:
                            _elu_into(o_t, h_sb, b, D)
                        elif layer == 1:
                            pre = op_.tile([P, D], dt.float32, tag="pre")
                            nc.vector.tensor_tensor(out=pre[:], in0=o_t[:],
                                                    in1=h_sb[:, b * D:(b + 1) * D],
                                                    op=mybir.AluOpType.add)
                            _elu_into(pre, h_sb, b, D)
                        else:
                            lg = op_.tile([P, CLS], dt.float32, tag="lg")
                            nc.vector.tensor_tensor(out=lg[:], in0=o_t[:],
                                                    in1=res_sb[:, b * CLS:(b + 1) * CLS],
                                                    op=mybir.AluOpType.add)
                            rows = P if b < NBT - 1 else LAST_ROWS
                            nc.sync.dma_start(outp[b * P:b * P + rows, :], lg[0:rows, :])

            if n_layers >= 0:
                project0()
            if n_layers >= 1:
                edge_phase(0)
            for layer in range(1, n_layers):
                feat_blocks(layer, 0, NBT)
                emit_ag(layer, 0)
                emit_ag(layer, 1)
                edge_phase(layer)

    nc.compile()
    return nc


LAST_RESULTS = None


def prepare(inputs):
    import os
    import ml_dtypes
    bf16 = ml_dtypes.bfloat16

    x = np.asarray(inputs["x"], np.float32)
    src = np.asarray(inputs["src"]).astype(np.int64)
    dst = np.asarray(inputs["dst"]).astype(np.int64)
    W1 = np.asarray(inputs["W1"], np.float32)
    W2 = np.asarray(inputs["W2"], np.float32)
    W3 = np.asarray(inputs["W3"], np.float32)
    res_W3 = np.asarray(inputs["res_W3"], np.float32)
    al1 = np.asarray(inputs["al1"], np.float32)
    ar1 = np.asarray(inputs["ar1"], np.float32)
    al2 = np.asarray(inputs["al2"], np.float32)
    ar2 = np.asarray(inputs["ar2"], np.float32)
    al3 = np.asarray(inputs["al3"], np.float32)
    ar3 = np.asarray(inputs["ar3"], np.float32)

    def ext(W, al, ar, nh, res=None):
        Wr = W.reshape(W.shape[0], nh, -1)
        wel = np.einsum("khf,hf->kh", Wr, al)
        wer = np.einsum("khf,hf->kh", Wr, ar)
        parts = [W, wel, wer] + ([res] if res is not None else [])
        return np.ascontiguousarray(np.concatenate(parts, axis=1), dtype=bf16)

    w1e = ext(W1, al1, ar1, H12)                 # [128, 264]
    w2e = ext(W2, al2, ar2, H12)                 # [256, 264]
    w3e = ext(W3, al3, ar3, 1, res_W3)           # [256, 130]

    tile_block, tile_reg, TT, idx16, dstpos = make_schedule(src, dst)
    nc = build_nc(tile_block, tile_reg, TT,
                  n_layers=int(os.environ.get("GAT_LAYERS", "3")))

    ident = np.eye(P, dtype=bf16)

    # region-major node permutation for the local layer-0 projection
    perm = np.concatenate(
        [np.arange(k * NLOC, k * NLOC + CJ[0]) for k in range(NC)]
        + [np.arange(k * NLOC + CJ[0], (k + 1) * NLOC) for k in range(NC)])
    xTfull = np.pad(x.T[:, perm], ((0, 0), (0, NTILE0 * P - N)))
    xTfull = np.ascontiguousarray(xTfull).astype(bf16)

    in_maps = []
    for k in range(NC):
        xk = x[k * NLOC:(k + 1) * NLOC].T                     # [128, 6250]
        xk = np.pad(xk, ((0, 0), (0, NBT * P - NLOC)))
        dp = dstpos[k]                                        # [128, TT]
        ohE = (dp[:, :, None] == np.arange(P, dtype=np.float32)[None, None, :])
        ohE = ohE.astype(bf16)                                # [e, t, d]
        ohT = np.ascontiguousarray(ohE.transpose(2, 1, 0))    # [d, t, e]
        in_maps.append({
            "xT": np.ascontiguousarray(xk).astype(bf16),
            "xTf": xTfull,
            "w1": w1e, "w2": w2e, "w3": w3e,
            "idx16": np.ascontiguousarray(idx16[k]),
            "ohE": np.ascontiguousarray(ohE.reshape(P, TT * P)),
            "ohT": ohT.reshape(P, TT * P),
            "ident": ident,
        })
    return nc, in_maps


def kernel(**inputs):
    from concourse.bass_utils import run_bass_kernel_spmd

    nc, in_maps = prepare(inputs)
    res = run_bass_kernel_spmd(nc, in_maps, core_ids=list(range(NC)))
    global LAST_RESULTS
    LAST_RESULTS = res
    out = np.concatenate([res.results[k]["out"] for k in range(NC)], axis=0)
    return out.astype(np.float32)
